# revision 57
# baseline (speedup 1.0000x reference)
"""Trainium2 Bass kernel for MultiHeadSelfAttention with ALiBi + adj bias.

fp8-DoubleRow design (B=2, L=2048, H=1024, NH=16, HS=64), 8 cores =
2 batches x 4 head-groups, heads {2g, 2g+1, 8+2g, 9+2g} per core.

Phase A (QKV proj): fp8 residual pairs. Host passes x1=fp8(xT),
x2=fp8(xT-x1), w1=fp8(64*W), w2=fp8(64*W-w1). psum = 64*xp via the 3
cross terms x1w1+x1w2+x2w1 as DoubleRow pairs over hidden k-tiles
(0.5 cyc/col, 6N vs 8N bf16). Evacuations build:
  qst[h] [128, 2, L] fp8: slot0=[Q1;Q2], slot1=[Q2;Q1] where
      Q1=fp8(sqrt2/64 * psum_q), Q2=fp8 residual (partition-stacked)
  kst[h] [128, L]  fp8: [K1;K2] likewise
  v_sb[t] [128, 4, 65] bf16: psum/64, col 64 = ones (memset)
spread over ACT (Q1/K1 copies), DVE (Q2/K2 stt), Pool (slot dups).

Phase B per (i-chunk, head): S^T+bias lands in ONE psum group per
(j, half): DoubleRow (kst-pair-broadcast x qst) gives all 4 fp8 cross
terms = 2*qk exactly; DoubleRow (id8 x cb-pair) adds 16*(gamma*adj +
alibi) via host-quantized hi+lo fp8 pair. exp on ACT with scale=1/16
-> E bf16. att@V bf16 (V stationary [128,65] with ones col -> row 64 =
denominators) lags 2 j-tiles so ACT never stalls. Normalize: DVE recip
+ Pool partition_broadcast + DVE mul -> attn bf16.

Phase C (out proj, bf16): shares the psS PSUM ring; i0's units are
interleaved into B(i1) head-blocks so ACT keeps running. yT f16
partials; host sums 4 cores per batch, transposes, adds out_bias.
"""

import numpy as np
import ml_dtypes
from contextlib import ExitStack

import concourse.tile as tile
from concourse import bacc, mybir
from concourse import bass_utils

F32 = mybir.dt.float32
F32R = mybir.dt.float32r
BF16 = mybir.dt.bfloat16
F16 = mybir.dt.float16
F8 = mybir.dt.float8e4
AF = mybir.ActivationFunctionType
DR = mybir.MatmulPerfMode.DoubleRow
AluOp = mybir.AluOpType
E4 = ml_dtypes.float8_e4m3

B, L, H, NH = 2, 2048, 1024, 16
HS = 64
NHL = 4            # local heads per core
P = 128
IC = 1024          # i-chunk width
NI = L // IC       # 2
NJ = L // P        # 16 j tiles
KT = H // P        # 8 hidden k-tiles
KP = KT // 2       # 4 k-tile pairs
WS = 64.0          # host weight scale for fp8
SQ2 = float(np.sqrt(2.0))
BS = 16.0          # bias scale inside psum (exp scale = 1/16)
QS = SQ2 / WS      # Q/K evac scale

RUN_KWARGS: dict = {}
_cache: dict = {}


def _build_program(with_qk_bias=False):
    nc = bacc.Bacc("TRN2", target_bir_lowering=False, debug=False,
                   enable_asserts=False, num_devices=8)

    # host pre-permutes to the SBUF pair layout [p, s, kp, c]
    x1 = nc.dram_tensor("x1", [P, 2, KP, L], F8, kind="ExternalInput").ap()
    x2 = nc.dram_tensor("x2", [P, 2, KP, L], F8, kind="ExternalInput").ap()
    wqk1 = nc.dram_tensor("wqk1", [P, 2, KP, 8 * HS], F8,
                          kind="ExternalInput").ap()
    wqk2 = nc.dram_tensor("wqk2", [P, 2, KP, 8 * HS], F8,
                          kind="ExternalInput").ap()
    wv1 = nc.dram_tensor("wv1", [P, 2, KP, NHL * HS], F8,
                         kind="ExternalInput").ap()
    wv2 = nc.dram_tensor("wv2", [P, 2, KP, NHL * HS], F8,
                         kind="ExternalInput").ap()
    cb8 = nc.dram_tensor("cb8", [NHL * NI * NJ * 2, P, IC], F8,
                         kind="ExternalInput").ap()
    id8 = nc.dram_tensor("id8", [P, 2, P], F8, kind="ExternalInput").ap()
    ow = nc.dram_tensor("ow", [2 * P, H], BF16, kind="ExternalInput").ap()
    if with_qk_bias:
        qkb = nc.dram_tensor("qkb", [1, 8 * HS], F32R,
                             kind="ExternalInput").ap()
        vb = nc.dram_tensor("vb", [1, NHL * HS], F32R,
                            kind="ExternalInput").ap()
        ones = nc.dram_tensor("ones", [1, IC], F32R,
                              kind="ExternalInput").ap()
    yT = nc.dram_tensor("yT", [H, L], F16, kind="ExternalOutput").ap()

    with tile.TileContext(nc) as tc, ExitStack() as ctx:
        persist = ctx.enter_context(tc.tile_pool(name="persist", bufs=1))
        qst = [persist.tile([P, 2, L], F8, tag=f"qst{h}", name=f"qst{h}")
               for h in range(NHL)]
        kst = [persist.tile([P, L], F8, tag=f"kst{h}", name=f"kst{h}")
               for h in range(NHL)]
        v_sb = [persist.tile([P, NHL, 65], BF16, tag=f"v{t}", name=f"v{t}")
                for t in range(NJ)]
        attn = [persist.tile([P, L], BF16, tag=f"attn{m}", name=f"attn{m}")
                for m in range(2)]
        id_sb = persist.tile([P, 2, P], F8, tag="id8")
        ow_sb = [persist.tile([P, H], BF16, tag=f"ow{k}", name=f"ow{k}")
                 for k in range(2)]
        if with_qk_bias:
            qkb_sb = persist.tile([1, 8 * HS], F32R, tag="qkb")
            nc.sync.dma_start(qkb_sb[:], qkb)
            vb_sb = persist.tile([1, NHL * HS], F32R, tag="vb")
            nc.sync.dma_start(vb_sb[:], vb)
            ones_sb = persist.tile([1, IC], F32R, tag="ones")
            nc.sync.dma_start(ones_sb[:], ones)
        # ones columns of v_sb (col 64 per head)
        for t in range(NJ):
            nc.vector.memset(v_sb[t][:, :, 64:65], 1.0)

        # cb pool lives outside Phase A so its tiles don't alias the xw
        # pool (address-reuse WAR would stall the cb DMAs until A ends)
        cbp = ctx.enter_context(tc.tile_pool(name="cbp", bufs=1))
        cb_tiles = {}

        def dma_cb(k, split=1):
            # iteration order is i-major: k = i*NHL + hl
            i, hl = divmod(k, NHL)
            t = cbp.tile([P, NJ * 2, IC], F8, tag="cb", bufs=2,
                         name=f"cbt{k % 3}")
            base = (hl * NI + i) * NJ * 2
            step = NJ * 2 // split
            for s in range(split):
                nc.sync.dma_start(
                    t[:, s * step:(s + 1) * step, :],
                    cb8[base + s * step:base + (s + 1) * step, :, :]
                    .transpose([1, 0, 2]))
            return t

        # ================= Phase A: QKV projection =================
        with tc.tile_pool(name="xw", bufs=1) as xw, \
             tc.tile_pool(name="psA", bufs=2, space="PSUM") as psA:
            # consolidated DMAs: one per weight tensor, one per (x, chunk)
            wq1a = xw.tile([P, 2, KP, 8 * HS], F8, tag="wq1", name="wq1a")
            wq2a = xw.tile([P, 2, KP, 8 * HS], F8, tag="wq2", name="wq2a")
            wv1a = xw.tile([P, 2, KP, NHL * HS], F8, tag="wv1", name="wv1a")
            wv2a = xw.tile([P, 2, KP, NHL * HS], F8, tag="wv2", name="wv2a")
            nc.sync.dma_start(wq1a[:], wqk1)
            nc.sync.dma_start(wq2a[:], wqk2)
            nc.sync.dma_start(id_sb[:], id8)
            xp1a = xw.tile([P, 2, KP, L], F8, tag="x1a", name="xp1a")
            xp2a = xw.tile([P, 2, KP, L], F8, tag="x2a", name="xp2a")
            nc.sync.dma_start(xp1a[:, :, :, 0:IC], x1[:, :, :, 0:IC])
            nc.sync.dma_start(xp2a[:, :, :, 0:IC], x2[:, :, :, 0:IC])
            # first half of cb(0) ahead of x-c1: lets (i0,h0) j0..7 start
            cb0 = cbp.tile([P, NJ * 2, IC], F8, tag="cb", bufs=2,
                           name="cbt0")
            nc.sync.dma_start(cb0[:, 0:NJ, :],
                              cb8[0:NJ, :, :].transpose([1, 0, 2]))
            nc.sync.dma_start(xp1a[:, :, :, IC:L], x1[:, :, :, IC:L])
            nc.sync.dma_start(xp2a[:, :, :, IC:L], x2[:, :, :, IC:L])
            nc.sync.dma_start(cb0[:, NJ:NJ * 2, :],
                              cb8[NJ:NJ * 2, :, :].transpose([1, 0, 2]))
            nc.sync.dma_start(wv1a[:], wv1)
            nc.sync.dma_start(wv2a[:], wv2)
            cb_tiles[0] = cb0
            cb_tiles[1] = dma_cb(1)
            for k in range(2):
                nc.sync.dma_start(ow_sb[k][:], ow[k * P:(k + 1) * P, :])

            def v_tile(t):
                tsl = slice(t * P, (t + 1) * P)
                vps = psA.tile([P, NHL, HS], F32, tag="vp", bufs=2)
                nmm = 0
                if with_qk_bias:
                    nc.tensor.matmul(vps[:], ones_sb[:, 0:P],
                                     vb_sb[:], start=True, stop=False)
                    nmm += 1
                for kp in range(KP):
                    for xs, w in ((xp1a, wv1a), (xp1a, wv2a), (xp2a, wv1a)):
                        nc.tensor.matmul(
                            vps[:], xs[:, :, kp, tsl], w[:, :, kp, :],
                            start=(nmm == 0), stop=(kp == KP - 1 and
                                                    xs is xp2a),
                            perf_mode=DR)
                        nmm += 1
                nc.vector.tensor_scalar_mul(v_sb[t][:, :, 0:HS], vps[:],
                                            1.0 / WS)

            # m-tiles: 0 = q(h0,h1), 1 = q(h2,h3), 2 = k(h0,h1), 3 = k(h2,h3)
            # order: k01, q01 first so B(h0) unblocks earliest; V tiles
            # interleave so the PE stream reaches B's S matmuls sooner
            nv = [0]
            for c in range(NI):
                csl = slice(c * IC, (c + 1) * IC)
                for m in (2, 0, 3, 1):
                    msl = slice(m * P, (m + 1) * P)
                    ps = psA.tile([P, IC], F32, tag="qkp", bufs=3)
                    nmm = [0, 0]
                    total = KP * 3 + (1 if with_qk_bias else 0)
                    if with_qk_bias:
                        for half in range(2):
                            hs_ = slice(half * 512, (half + 1) * 512)
                            nc.tensor.matmul(ps[:, hs_], qkb_sb[:, msl],
                                             ones_sb[:, 0:512],
                                             start=True, stop=False)
                            nmm[half] += 1
                    # term order (wq1,xp1),(wq1,xp2),(wq2,xp1): 2 ldw/kp
                    for kp in range(KP):
                        for w, xs in ((wq1a, xp1a), (wq1a, xp2a),
                                      (wq2a, xp1a)):
                            lhs = w[:, :, kp, msl]
                            for half in range(2):
                                hs_ = slice(half * 512, (half + 1) * 512)
                                xsl = slice(c * IC + half * 512,
                                            c * IC + half * 512 + 512)
                                nc.tensor.matmul(
                                    ps[:, hs_], lhs, xs[:, :, kp, xsl],
                                    start=(nmm[half] == 0),
                                    stop=(nmm[half] == total - 1),
                                    perf_mode=DR)
                                nmm[half] += 1
                    # evacuate this (m, c) psum
                    for hl in range(2):
                        h = (m % 2) * 2 + hl
                        rows = slice(hl * HS, hl * HS + HS)
                        if m in (0, 1):   # q -> qst[h]
                            # Q1 -> (0:64, s0)  [ACT]
                            nc.scalar.activation(qst[h][0:HS, 0, csl],
                                                 ps[rows, :], AF.Copy,
                                                 scale=QS)
                            # Q2 -> (64:128, s0) [DVE]
                            nc.vector.scalar_tensor_tensor(
                                qst[h][HS:P, 0, csl], ps[rows, :], QS,
                                qst[h][0:HS, 0, csl], AluOp.mult,
                                AluOp.subtract)
                            # dups -> slot1 [one on ACT, one on Pool so
                            # neither engine serializes the A pipeline]
                            nc.gpsimd.tensor_copy(qst[h][0:HS, 1, csl],
                                                  qst[h][HS:P, 0, csl])
                            nc.gpsimd.tensor_copy(qst[h][HS:P, 1, csl],
                                                  qst[h][0:HS, 0, csl])
                        else:             # k -> kst[h]
                            nc.scalar.activation(kst[h][0:HS, csl],
                                                 ps[rows, :], AF.Copy,
                                                 scale=QS)
                            nc.vector.scalar_tensor_tensor(
                                kst[h][HS:P, csl], ps[rows, :], QS,
                                kst[h][0:HS, csl], AluOp.mult,
                                AluOp.subtract)
                    # 2 V tiles per (m, c) unit -> all 16 done with A
                    for _ in range(2):
                        if nv[0] < NJ:
                            v_tile(nv[0])
                            nv[0] += 1
            while nv[0] < NJ:
                v_tile(nv[0])
                nv[0] += 1

        # ================= Phase B + C =================
        with tc.tile_pool(name="cbp", bufs=1) as cbp, \
             tc.tile_pool(name="e_pool", bufs=4) as e_pool, \
             tc.tile_pool(name="r_pool", bufs=2) as r_pool, \
             tc.tile_pool(name="y_pool", bufs=3) as y_pool, \
             tc.tile_pool(name="psS", bufs=3, space="PSUM") as psS, \
             tc.tile_pool(name="psO", bufs=1, space="PSUM") as psO:

            n_c = [0]

            def c_unit(m, n, src_i):
                msl = slice(m * P, (m + 1) * P)
                ps = psS.tile([P, IC], F32, tag="sp")
                for kt in range(2):
                    for half in range(2):
                        hs_ = slice(half * 512, (half + 1) * 512)
                        asl = slice(src_i * IC + half * 512,
                                    src_i * IC + half * 512 + 512)
                        nc.tensor.matmul(ps[:, hs_], ow_sb[kt][:, msl],
                                         attn[kt][:, asl],
                                         start=(kt == 0), stop=(kt == 1))
                yt = y_pool.tile([P, IC], F16, tag="yt")
                # tail units (src_i==1) may use the then-idle ACT engine;
                # interleaved i0 units must not steal ACT from exp
                if src_i == 1 and n_c[0] % 2 == 1:
                    nc.scalar.activation(yt[:], ps[:], AF.Copy)
                else:
                    nc.vector.tensor_copy(yt[:], ps[:])
                n_c[0] += 1
                nc.sync.dma_start(
                    yT[msl, src_i * IC:(src_i + 1) * IC], yt[:])

            for i in range(NI):
                isl = slice(i * IC, (i + 1) * IC)
                for hl in range(NHL):
                    k = i * NHL + hl
                    if k + 2 < NI * NHL:
                        cb_tiles[k + 2] = dma_cb(k + 2)
                    cb_t = cb_tiles.pop(k)
                    c_queue = ([(lambda m=m, n=n: c_unit(m, n, 0))
                                for m in (2 * hl, 2 * hl + 1)
                                for n in range(NI)] if i == 1 else [])
                    avp = psO.tile([65, IC], F32, tag="avp")
                    ets = {}

                    def attv(j):
                        et = ets.pop(j)
                        for half in range(2):
                            hs_ = slice(half * 512, (half + 1) * 512)
                            nc.tensor.matmul(
                                avp[:, hs_], v_sb[j][:, hl, :], et[:, hs_],
                                start=(j == 0), stop=(j == NJ - 1))

                    for j in range(NJ):
                        sp = psS.tile([P, IC], F32, tag="sp")
                        kpair = (kst[hl][:, j * P:(j + 1) * P]
                                 .unsqueeze(1).broadcast_to([P, 2, P]))
                        # both S halves first, then both bias halves:
                        # 2 ldweights per j instead of 4
                        for half in range(2):
                            hs_ = slice(half * 512, (half + 1) * 512)
                            qsl = slice(i * IC + half * 512,
                                        i * IC + half * 512 + 512)
                            nc.tensor.matmul(sp[:, hs_], kpair,
                                             qst[hl][:, :, qsl],
                                             start=True, stop=False,
                                             perf_mode=DR)
                        for half in range(2):
                            hs_ = slice(half * 512, (half + 1) * 512)
                            nc.tensor.matmul(
                                sp[:, hs_], id_sb[:],
                                cb_t[:, 2 * j:2 * j + 2,
                                     half * 512:half * 512 + 512],
                                start=False, stop=True, perf_mode=DR)
                        et = e_pool.tile([P, IC], BF16, tag="et")
                        nc.scalar.activation(et[:], sp[:], AF.Exp,
                                             scale=1.0 / BS)
                        ets[j] = et
                        if j >= 2:
                            attv(j - 2)
                        # spread C(i0) units through B(i1) j-loops so the
                        # PE-side boundary clump doesn't starve ACT
                        if j in (5, 9, 13) and c_queue:
                            c_queue.pop(0)()
                    attv(NJ - 2)
                    attv(NJ - 1)

                    # normalize -> attn bf16. Copy avp out first so the
                    # next head's attV group isn't blocked on the whole
                    # recip->bcast->mul chain (psO has a single buffer).
                    last = (i == NI - 1 and hl == NHL - 1)
                    if not last:
                        avs = r_pool.tile([65, IC], F32, tag="avs",
                                          bufs=1)
                        nc.vector.tensor_copy(avs[:], avp[:])
                    else:
                        avs = avp
                    rt = r_pool.tile([1, IC], F32R, tag="rt")
                    with nc.allow_low_precision(reason="f32r denom recip"):
                        nc.vector.reciprocal(rt[:], avs[64:65, :])
                    rbs = r_pool.tile([HS, IC], F32, tag="rbs")
                    nc.gpsimd.partition_broadcast(rbs[:],
                                                  rt[:].bitcast(F32))
                    hp = (hl % 2) * HS
                    nc.vector.tensor_mul(
                        attn[hl // 2][hp:hp + HS, isl], avs[0:HS, :],
                        rbs[:])

                    # drain any remaining C(i0) units for this head-block
                    while c_queue:
                        c_queue.pop(0)()
            for m in range(H // P):
                for n in range(NI):
                    c_unit(m, n, 1)

    nc.compile()
    return nc


def _alibi_slopes():
    n = NH // 2
    start = 2.0 ** (-(2.0 ** (-(np.log2(n) - 3.0))))
    s = np.array([start * start ** i for i in range(n)], dtype=np.float32)
    return np.concatenate([s, np.zeros(n, dtype=np.float32)])


def _q8(a):
    return np.clip(a, -240.0, 240.0).astype(E4)


def _build_in_maps(x, adj, weights, in_bias, gamma, out_w, with_qk_bias):
    from concurrent.futures import ThreadPoolExecutor
    slopes = _alibi_slopes()
    ar = np.arange(L, dtype=np.float32)
    dist = -np.abs(ar[None, :] - ar[:, None])
    idm = np.zeros((P, 2, P), dtype=E4)
    idm[:, 0][np.arange(P), np.arange(P)] = 1.0
    idm[:, 1][np.arange(P), np.arange(P)] = 1.0
    adjT_by_b = [np.ascontiguousarray(adj[b, 0].T) for b in range(B)]
    xT_by_b = [np.ascontiguousarray(x[b].T) for b in range(B)]
    def _pairize(a):
        # [H, C] -> [P, 2, KP, C]: row (2*kp+s)*P + p -> (p, s, kp)
        c = a.shape[1]
        return np.ascontiguousarray(
            a.reshape(KP, 2, P, c).transpose(2, 1, 0, 3))

    x1_by_b, x2_by_b = [], []
    for b in range(B):
        x1b = _q8(xT_by_b[b])
        x2b = _q8(xT_by_b[b] - x1b.astype(np.float32))
        x1_by_b.append(_pairize(x1b))
        x2_by_b.append(_pairize(x2b))

    def _make_cb(core):
        b, g = divmod(core, 4)
        heads = [2 * g, 2 * g + 1, 8 + 2 * g, 9 + 2 * g]
        out = np.empty((NHL * NI * NJ * 2, P, IC), dtype=E4)
        for hl, hh in enumerate(heads):
            t = BS * gamma[0, hh, 0, 0] * adjT_by_b[b]
            if slopes[hh] != 0.0:
                t = t + (BS * slopes[hh]) * dist
            for i in range(NI):
                blk = t[:, i * IC:(i + 1) * IC].reshape(NJ, P, IC)
                hi = _q8(blk)
                lo = _q8(blk - hi.astype(np.float32))
                base = (hl * NI + i) * NJ * 2
                out[base + 0:base + 2 * NJ:2] = hi
                out[base + 1:base + 2 * NJ:2] = lo
        return out

    with ThreadPoolExecutor(max_workers=8) as ex:
        cb_by_core = list(ex.map(_make_cb, range(8)))

    in_maps = []
    for core in range(8):
        b, g = divmod(core, 4)
        heads = [2 * g, 2 * g + 1, 8 + 2 * g, 9 + 2 * g]
        qcols = np.concatenate([np.arange(192 * h, 192 * h + 64)
                                for h in heads])
        kcols = qcols + 64
        vcols = qcols + 128
        wqk = np.ascontiguousarray(
            weights[:, np.concatenate([qcols, kcols])]) * WS
        wqk1 = _pairize(_q8(wqk))
        wqk2 = _pairize(_q8(wqk - _q8(wqk).astype(np.float32)))
        wv = np.ascontiguousarray(weights[:, vcols]) * WS
        wv1 = _pairize(_q8(wv))
        wv2 = _pairize(_q8(wv - _q8(wv).astype(np.float32)))
        owm = np.ascontiguousarray(
            out_w[np.concatenate([np.arange(64 * h, 64 * h + 64)
                                  for h in heads]), :]).astype(
                                      ml_dtypes.bfloat16)
        m = {
            "x1": x1_by_b[b], "x2": x2_by_b[b],
            "wqk1": wqk1, "wqk2": wqk2, "wv1": wv1, "wv2": wv2,
            "cb8": cb_by_core[core], "id8": idm, "ow": owm,
        }
        if with_qk_bias:
            m["qkb"] = np.ascontiguousarray(
                in_bias[0, 0, np.concatenate([qcols, kcols])].reshape(1, -1)
            ).astype(np.float32) * WS
            m["vb"] = np.ascontiguousarray(
                in_bias[0, 0, vcols].reshape(1, -1)).astype(np.float32) * WS
            m["ones"] = np.ones((1, IC), dtype=np.float32)
        in_maps.append(m)
    return in_maps


def kernel(x, adj, weights, in_bias, out_w, out_bias, gamma):
    x = np.asarray(x, dtype=np.float32)
    adj = np.asarray(adj, dtype=np.float32)
    weights = np.asarray(weights, dtype=np.float32)
    in_bias = np.asarray(in_bias, dtype=np.float32)
    out_w = np.asarray(out_w, dtype=np.float32)
    out_bias = np.asarray(out_bias, dtype=np.float32)
    gamma = np.asarray(gamma, dtype=np.float32)

    with_qk_bias = bool(np.any(in_bias[0, 0, :]))
    key = f"nc_{with_qk_bias}"
    if key not in _cache:
        _cache[key] = _build_program(with_qk_bias)
    nc = _cache[key]

    in_maps = _build_in_maps(x, adj, weights, in_bias, gamma, out_w,
                             with_qk_bias)
    res = bass_utils.run_bass_kernel_spmd(nc, in_maps,
                                          core_ids=list(range(8)),
                                          **RUN_KWARGS)
    _cache["last_result"] = res

    out = np.empty((B, L, H), dtype=np.float32)
    for b in range(B):
        acc = res.results[4 * b]["yT"].astype(np.float32)
        for g in range(1, 4):
            acc += res.results[4 * b + g]["yT"].astype(np.float32)
        out[b] = acc.T + out_bias[0, 0][None, :]
    return out


# revision 58
# speedup vs baseline: 1.0002x; 1.0002x over previous
"""Trainium2 Bass kernel for MultiHeadSelfAttention with ALiBi + adj bias.

fp8-DoubleRow design (B=2, L=2048, H=1024, NH=16, HS=64), 8 cores =
2 batches x 4 head-groups, heads {2g, 2g+1, 8+2g, 9+2g} per core.

Phase A (QKV proj): fp8 residual pairs. Host passes x1=fp8(xT),
x2=fp8(xT-x1), w1=fp8(64*W), w2=fp8(64*W-w1). psum = 64*xp via the 3
cross terms x1w1+x1w2+x2w1 as DoubleRow pairs over hidden k-tiles
(0.5 cyc/col, 6N vs 8N bf16). Evacuations build:
  qst[h] [128, 2, L] fp8: slot0=[Q1;Q2], slot1=[Q2;Q1] where
      Q1=fp8(sqrt2/64 * psum_q), Q2=fp8 residual (partition-stacked)
  kst[h] [128, L]  fp8: [K1;K2] likewise
  v_sb[t] [128, 4, 65] bf16: psum/64, col 64 = ones (memset)
spread over ACT (Q1/K1 copies), DVE (Q2/K2 stt), Pool (slot dups).

Phase B per (i-chunk, head): S^T+bias lands in ONE psum group per
(j, half): DoubleRow (kst-pair-broadcast x qst) gives all 4 fp8 cross
terms = 2*qk exactly; DoubleRow (id8 x cb-pair) adds 16*(gamma*adj +
alibi) via host-quantized hi+lo fp8 pair. exp on ACT with scale=1/16
-> E bf16. att@V bf16 (V stationary [128,65] with ones col -> row 64 =
denominators) lags 2 j-tiles so ACT never stalls. Normalize: DVE recip
+ Pool partition_broadcast + DVE mul -> attn bf16.

Phase C (out proj, bf16): shares the psS PSUM ring; i0's units are
interleaved into B(i1) head-blocks so ACT keeps running. yT f16
partials; host sums 4 cores per batch, transposes, adds out_bias.
"""

import numpy as np
import ml_dtypes
from contextlib import ExitStack

import concourse.tile as tile
from concourse import bacc, mybir
from concourse import bass_utils

F32 = mybir.dt.float32
F32R = mybir.dt.float32r
BF16 = mybir.dt.bfloat16
F16 = mybir.dt.float16
F8 = mybir.dt.float8e4
AF = mybir.ActivationFunctionType
DR = mybir.MatmulPerfMode.DoubleRow
AluOp = mybir.AluOpType
E4 = ml_dtypes.float8_e4m3

B, L, H, NH = 2, 2048, 1024, 16
HS = 64
NHL = 4            # local heads per core
P = 128
IC = 1024          # i-chunk width
NI = L // IC       # 2
NJ = L // P        # 16 j tiles
KT = H // P        # 8 hidden k-tiles
KP = KT // 2       # 4 k-tile pairs
WS = 64.0          # host weight scale for fp8
SQ2 = float(np.sqrt(2.0))
BS = 16.0          # bias scale inside psum (exp scale = 1/16)
QS = SQ2 / WS      # Q/K evac scale

RUN_KWARGS: dict = {}
_cache: dict = {}


def _build_program(with_qk_bias=False):
    nc = bacc.Bacc("TRN2", target_bir_lowering=False, debug=False,
                   enable_asserts=False, num_devices=8)

    # host pre-permutes to the SBUF pair layout [p, s, kp, c]
    x1 = nc.dram_tensor("x1", [P, 2, KP, L], F8, kind="ExternalInput").ap()
    x2 = nc.dram_tensor("x2", [P, 2, KP, L], F8, kind="ExternalInput").ap()
    wqk1 = nc.dram_tensor("wqk1", [P, 2, KP, 8 * HS], F8,
                          kind="ExternalInput").ap()
    wqk2 = nc.dram_tensor("wqk2", [P, 2, KP, 8 * HS], F8,
                          kind="ExternalInput").ap()
    wv1 = nc.dram_tensor("wv1", [P, 2, KP, NHL * HS], F8,
                         kind="ExternalInput").ap()
    wv2 = nc.dram_tensor("wv2", [P, 2, KP, NHL * HS], F8,
                         kind="ExternalInput").ap()
    cb8 = nc.dram_tensor("cb8", [NHL * NI * NJ * 2, P, IC], F8,
                         kind="ExternalInput").ap()
    id8 = nc.dram_tensor("id8", [P, 2, P], F8, kind="ExternalInput").ap()
    ow = nc.dram_tensor("ow", [2 * P, H], BF16, kind="ExternalInput").ap()
    if with_qk_bias:
        qkb = nc.dram_tensor("qkb", [1, 8 * HS], F32R,
                             kind="ExternalInput").ap()
        vb = nc.dram_tensor("vb", [1, NHL * HS], F32R,
                            kind="ExternalInput").ap()
        ones = nc.dram_tensor("ones", [1, IC], F32R,
                              kind="ExternalInput").ap()
    yT = nc.dram_tensor("yT", [H, L], F16, kind="ExternalOutput").ap()

    with tile.TileContext(nc) as tc, ExitStack() as ctx:
        persist = ctx.enter_context(tc.tile_pool(name="persist", bufs=1))
        qst = [persist.tile([P, 2, L], F8, tag=f"qst{h}", name=f"qst{h}")
               for h in range(NHL)]
        kst = [persist.tile([P, L], F8, tag=f"kst{h}", name=f"kst{h}")
               for h in range(NHL)]
        v_sb = [persist.tile([P, NHL, 65], BF16, tag=f"v{t}", name=f"v{t}")
                for t in range(NJ)]
        attn = [persist.tile([P, L], BF16, tag=f"attn{m}", name=f"attn{m}")
                for m in range(2)]
        id_sb = persist.tile([P, 2, P], F8, tag="id8")
        ow_sb = [persist.tile([P, H], BF16, tag=f"ow{k}", name=f"ow{k}")
                 for k in range(2)]
        if with_qk_bias:
            qkb_sb = persist.tile([1, 8 * HS], F32R, tag="qkb")
            nc.sync.dma_start(qkb_sb[:], qkb)
            vb_sb = persist.tile([1, NHL * HS], F32R, tag="vb")
            nc.sync.dma_start(vb_sb[:], vb)
            ones_sb = persist.tile([1, IC], F32R, tag="ones")
            nc.sync.dma_start(ones_sb[:], ones)
        # ones columns of v_sb (col 64 per head)
        for t in range(NJ):
            nc.vector.memset(v_sb[t][:, :, 64:65], 1.0)

        # cb pool lives outside Phase A so its tiles don't alias the xw
        # pool (address-reuse WAR would stall the cb DMAs until A ends)
        cbp = ctx.enter_context(tc.tile_pool(name="cbp", bufs=1))
        cb_tiles = {}

        def dma_cb(k, split=1):
            # iteration order is i-major: k = i*NHL + hl
            i, hl = divmod(k, NHL)
            t = cbp.tile([P, NJ * 2, IC], F8, tag="cb", bufs=2,
                         name=f"cbt{k % 3}")
            base = (hl * NI + i) * NJ * 2
            step = NJ * 2 // split
            for s in range(split):
                nc.sync.dma_start(
                    t[:, s * step:(s + 1) * step, :],
                    cb8[base + s * step:base + (s + 1) * step, :, :]
                    .transpose([1, 0, 2]))
            return t

        # ================= Phase A: QKV projection =================
        with tc.tile_pool(name="xw", bufs=1) as xw, \
             tc.tile_pool(name="psA", bufs=2, space="PSUM") as psA:
            # consolidated DMAs: one per weight tensor, one per (x, chunk)
            wq1a = xw.tile([P, 2, KP, 8 * HS], F8, tag="wq1", name="wq1a")
            wq2a = xw.tile([P, 2, KP, 8 * HS], F8, tag="wq2", name="wq2a")
            wv1a = xw.tile([P, 2, KP, NHL * HS], F8, tag="wv1", name="wv1a")
            wv2a = xw.tile([P, 2, KP, NHL * HS], F8, tag="wv2", name="wv2a")
            nc.sync.dma_start(wq1a[:], wqk1)
            nc.sync.dma_start(wq2a[:], wqk2)
            nc.sync.dma_start(id_sb[:], id8)
            xp1a = xw.tile([P, 2, KP, L], F8, tag="x1a", name="xp1a")
            xp2a = xw.tile([P, 2, KP, L], F8, tag="x2a", name="xp2a")
            nc.sync.dma_start(xp1a[:, :, :, 0:IC], x1[:, :, :, 0:IC])
            nc.sync.dma_start(xp2a[:, :, :, 0:IC], x2[:, :, :, 0:IC])
            # first half of cb(0) ahead of x-c1: lets (i0,h0) j0..7 start
            cb0 = cbp.tile([P, NJ * 2, IC], F8, tag="cb", bufs=2,
                           name="cbt0")
            nc.sync.dma_start(cb0[:, 0:NJ, :],
                              cb8[0:NJ, :, :].transpose([1, 0, 2]))
            nc.sync.dma_start(xp1a[:, :, :, IC:L], x1[:, :, :, IC:L])
            nc.sync.dma_start(xp2a[:, :, :, IC:L], x2[:, :, :, IC:L])
            nc.sync.dma_start(cb0[:, NJ:NJ * 2, :],
                              cb8[NJ:NJ * 2, :, :].transpose([1, 0, 2]))
            nc.sync.dma_start(wv1a[:], wv1)
            nc.sync.dma_start(wv2a[:], wv2)
            cb_tiles[0] = cb0
            cb_tiles[1] = dma_cb(1, split=2)
            for k in range(2):
                nc.sync.dma_start(ow_sb[k][:], ow[k * P:(k + 1) * P, :])

            def v_tile(t):
                tsl = slice(t * P, (t + 1) * P)
                vps = psA.tile([P, NHL, HS], F32, tag="vp", bufs=2)
                nmm = 0
                if with_qk_bias:
                    nc.tensor.matmul(vps[:], ones_sb[:, 0:P],
                                     vb_sb[:], start=True, stop=False)
                    nmm += 1
                for kp in range(KP):
                    for xs, w in ((xp1a, wv1a), (xp1a, wv2a), (xp2a, wv1a)):
                        nc.tensor.matmul(
                            vps[:], xs[:, :, kp, tsl], w[:, :, kp, :],
                            start=(nmm == 0), stop=(kp == KP - 1 and
                                                    xs is xp2a),
                            perf_mode=DR)
                        nmm += 1
                nc.vector.tensor_scalar_mul(v_sb[t][:, :, 0:HS], vps[:],
                                            1.0 / WS)

            # m-tiles: 0 = q(h0,h1), 1 = q(h2,h3), 2 = k(h0,h1), 3 = k(h2,h3)
            # order: k01, q01 first so B(h0) unblocks earliest; V tiles
            # interleave so the PE stream reaches B's S matmuls sooner
            nv = [0]
            for c in range(NI):
                csl = slice(c * IC, (c + 1) * IC)
                for m in (2, 0, 3, 1):
                    msl = slice(m * P, (m + 1) * P)
                    ps = psA.tile([P, IC], F32, tag="qkp", bufs=3)
                    nmm = [0, 0]
                    total = KP * 3 + (1 if with_qk_bias else 0)
                    if with_qk_bias:
                        for half in range(2):
                            hs_ = slice(half * 512, (half + 1) * 512)
                            nc.tensor.matmul(ps[:, hs_], qkb_sb[:, msl],
                                             ones_sb[:, 0:512],
                                             start=True, stop=False)
                            nmm[half] += 1
                    # term order (wq1,xp1),(wq1,xp2),(wq2,xp1): 2 ldw/kp
                    for kp in range(KP):
                        for w, xs in ((wq1a, xp1a), (wq1a, xp2a),
                                      (wq2a, xp1a)):
                            lhs = w[:, :, kp, msl]
                            for half in range(2):
                                hs_ = slice(half * 512, (half + 1) * 512)
                                xsl = slice(c * IC + half * 512,
                                            c * IC + half * 512 + 512)
                                nc.tensor.matmul(
                                    ps[:, hs_], lhs, xs[:, :, kp, xsl],
                                    start=(nmm[half] == 0),
                                    stop=(nmm[half] == total - 1),
                                    perf_mode=DR)
                                nmm[half] += 1
                    # evacuate this (m, c) psum
                    for hl in range(2):
                        h = (m % 2) * 2 + hl
                        rows = slice(hl * HS, hl * HS + HS)
                        if m in (0, 1):   # q -> qst[h]
                            # Q1 -> (0:64, s0)  [ACT]
                            nc.scalar.activation(qst[h][0:HS, 0, csl],
                                                 ps[rows, :], AF.Copy,
                                                 scale=QS)
                            # Q2 -> (64:128, s0) [DVE]
                            nc.vector.scalar_tensor_tensor(
                                qst[h][HS:P, 0, csl], ps[rows, :], QS,
                                qst[h][0:HS, 0, csl], AluOp.mult,
                                AluOp.subtract)
                            # dups -> slot1 [one on ACT, one on Pool so
                            # neither engine serializes the A pipeline]
                            nc.gpsimd.tensor_copy(qst[h][0:HS, 1, csl],
                                                  qst[h][HS:P, 0, csl])
                            nc.gpsimd.tensor_copy(qst[h][HS:P, 1, csl],
                                                  qst[h][0:HS, 0, csl])
                        else:             # k -> kst[h]
                            nc.scalar.activation(kst[h][0:HS, csl],
                                                 ps[rows, :], AF.Copy,
                                                 scale=QS)
                            nc.vector.scalar_tensor_tensor(
                                kst[h][HS:P, csl], ps[rows, :], QS,
                                kst[h][0:HS, csl], AluOp.mult,
                                AluOp.subtract)
                    # 2 V tiles per (m, c) unit -> all 16 done with A
                    for _ in range(2):
                        if nv[0] < NJ:
                            v_tile(nv[0])
                            nv[0] += 1
            while nv[0] < NJ:
                v_tile(nv[0])
                nv[0] += 1

        # ================= Phase B + C =================
        with tc.tile_pool(name="cbp", bufs=1) as cbp, \
             tc.tile_pool(name="e_pool", bufs=4) as e_pool, \
             tc.tile_pool(name="r_pool", bufs=2) as r_pool, \
             tc.tile_pool(name="y_pool", bufs=3) as y_pool, \
             tc.tile_pool(name="psS", bufs=3, space="PSUM") as psS, \
             tc.tile_pool(name="psO", bufs=1, space="PSUM") as psO:

            n_c = [0]

            def c_unit(m, n, src_i):
                msl = slice(m * P, (m + 1) * P)
                ps = psS.tile([P, IC], F32, tag="sp")
                for kt in range(2):
                    for half in range(2):
                        hs_ = slice(half * 512, (half + 1) * 512)
                        asl = slice(src_i * IC + half * 512,
                                    src_i * IC + half * 512 + 512)
                        nc.tensor.matmul(ps[:, hs_], ow_sb[kt][:, msl],
                                         attn[kt][:, asl],
                                         start=(kt == 0), stop=(kt == 1))
                yt = y_pool.tile([P, IC], F16, tag="yt")
                # tail units (src_i==1) may use the then-idle ACT engine;
                # interleaved i0 units must not steal ACT from exp
                if src_i == 1 and n_c[0] % 2 == 1:
                    nc.scalar.activation(yt[:], ps[:], AF.Copy)
                else:
                    nc.vector.tensor_copy(yt[:], ps[:])
                n_c[0] += 1
                nc.sync.dma_start(
                    yT[msl, src_i * IC:(src_i + 1) * IC], yt[:])

            for i in range(NI):
                isl = slice(i * IC, (i + 1) * IC)
                for hl in range(NHL):
                    k = i * NHL + hl
                    if k + 2 < NI * NHL:
                        cb_tiles[k + 2] = dma_cb(k + 2)
                    cb_t = cb_tiles.pop(k)
                    c_queue = ([(lambda m=m, n=n: c_unit(m, n, 0))
                                for m in (2 * hl, 2 * hl + 1)
                                for n in range(NI)] if i == 1 else [])
                    avp = psO.tile([65, IC], F32, tag="avp")
                    ets = {}

                    def attv(j):
                        et = ets.pop(j)
                        for half in range(2):
                            hs_ = slice(half * 512, (half + 1) * 512)
                            nc.tensor.matmul(
                                avp[:, hs_], v_sb[j][:, hl, :], et[:, hs_],
                                start=(j == 0), stop=(j == NJ - 1))

                    for j in range(NJ):
                        sp = psS.tile([P, IC], F32, tag="sp")
                        kpair = (kst[hl][:, j * P:(j + 1) * P]
                                 .unsqueeze(1).broadcast_to([P, 2, P]))
                        # both S halves first, then both bias halves:
                        # 2 ldweights per j instead of 4
                        for half in range(2):
                            hs_ = slice(half * 512, (half + 1) * 512)
                            qsl = slice(i * IC + half * 512,
                                        i * IC + half * 512 + 512)
                            nc.tensor.matmul(sp[:, hs_], kpair,
                                             qst[hl][:, :, qsl],
                                             start=True, stop=False,
                                             perf_mode=DR)
                        for half in range(2):
                            hs_ = slice(half * 512, (half + 1) * 512)
                            nc.tensor.matmul(
                                sp[:, hs_], id_sb[:],
                                cb_t[:, 2 * j:2 * j + 2,
                                     half * 512:half * 512 + 512],
                                start=False, stop=True, perf_mode=DR)
                        et = e_pool.tile([P, IC], BF16, tag="et")
                        nc.scalar.activation(et[:], sp[:], AF.Exp,
                                             scale=1.0 / BS)
                        ets[j] = et
                        if j >= 2:
                            attv(j - 2)
                        # spread C(i0) units through B(i1) j-loops so the
                        # PE-side boundary clump doesn't starve ACT
                        if j in (5, 9, 13) and c_queue:
                            c_queue.pop(0)()
                    attv(NJ - 2)
                    attv(NJ - 1)

                    # normalize -> attn bf16. Copy avp out first so the
                    # next head's attV group isn't blocked on the whole
                    # recip->bcast->mul chain (psO has a single buffer).
                    last = (i == NI - 1 and hl == NHL - 1)
                    if not last:
                        avs = r_pool.tile([65, IC], F32, tag="avs",
                                          bufs=1)
                        nc.vector.tensor_copy(avs[:], avp[:])
                    else:
                        avs = avp
                    rt = r_pool.tile([1, IC], F32R, tag="rt")
                    with nc.allow_low_precision(reason="f32r denom recip"):
                        nc.vector.reciprocal(rt[:], avs[64:65, :])
                    rbs = r_pool.tile([HS, IC], F32, tag="rbs")
                    nc.gpsimd.partition_broadcast(rbs[:],
                                                  rt[:].bitcast(F32))
                    hp = (hl % 2) * HS
                    nc.vector.tensor_mul(
                        attn[hl // 2][hp:hp + HS, isl], avs[0:HS, :],
                        rbs[:])

                    # drain any remaining C(i0) units for this head-block
                    while c_queue:
                        c_queue.pop(0)()
            for m in range(H // P):
                for n in range(NI):
                    c_unit(m, n, 1)

    nc.compile()
    return nc


def _alibi_slopes():
    n = NH // 2
    start = 2.0 ** (-(2.0 ** (-(np.log2(n) - 3.0))))
    s = np.array([start * start ** i for i in range(n)], dtype=np.float32)
    return np.concatenate([s, np.zeros(n, dtype=np.float32)])


def _q8(a):
    return np.clip(a, -240.0, 240.0).astype(E4)


def _build_in_maps(x, adj, weights, in_bias, gamma, out_w, with_qk_bias):
    from concurrent.futures import ThreadPoolExecutor
    slopes = _alibi_slopes()
    ar = np.arange(L, dtype=np.float32)
    dist = -np.abs(ar[None, :] - ar[:, None])
    idm = np.zeros((P, 2, P), dtype=E4)
    idm[:, 0][np.arange(P), np.arange(P)] = 1.0
    idm[:, 1][np.arange(P), np.arange(P)] = 1.0
    adjT_by_b = [np.ascontiguousarray(adj[b, 0].T) for b in range(B)]
    xT_by_b = [np.ascontiguousarray(x[b].T) for b in range(B)]
    def _pairize(a):
        # [H, C] -> [P, 2, KP, C]: row (2*kp+s)*P + p -> (p, s, kp)
        c = a.shape[1]
        return np.ascontiguousarray(
            a.reshape(KP, 2, P, c).transpose(2, 1, 0, 3))

    x1_by_b, x2_by_b = [], []
    for b in range(B):
        x1b = _q8(xT_by_b[b])
        x2b = _q8(xT_by_b[b] - x1b.astype(np.float32))
        x1_by_b.append(_pairize(x1b))
        x2_by_b.append(_pairize(x2b))

    def _make_cb(core):
        b, g = divmod(core, 4)
        heads = [2 * g, 2 * g + 1, 8 + 2 * g, 9 + 2 * g]
        out = np.empty((NHL * NI * NJ * 2, P, IC), dtype=E4)
        for hl, hh in enumerate(heads):
            t = BS * gamma[0, hh, 0, 0] * adjT_by_b[b]
            if slopes[hh] != 0.0:
                t = t + (BS * slopes[hh]) * dist
            for i in range(NI):
                blk = t[:, i * IC:(i + 1) * IC].reshape(NJ, P, IC)
                hi = _q8(blk)
                lo = _q8(blk - hi.astype(np.float32))
                base = (hl * NI + i) * NJ * 2
                out[base + 0:base + 2 * NJ:2] = hi
                out[base + 1:base + 2 * NJ:2] = lo
        return out

    with ThreadPoolExecutor(max_workers=8) as ex:
        cb_by_core = list(ex.map(_make_cb, range(8)))

    in_maps = []
    for core in range(8):
        b, g = divmod(core, 4)
        heads = [2 * g, 2 * g + 1, 8 + 2 * g, 9 + 2 * g]
        qcols = np.concatenate([np.arange(192 * h, 192 * h + 64)
                                for h in heads])
        kcols = qcols + 64
        vcols = qcols + 128
        wqk = np.ascontiguousarray(
            weights[:, np.concatenate([qcols, kcols])]) * WS
        wqk1 = _pairize(_q8(wqk))
        wqk2 = _pairize(_q8(wqk - _q8(wqk).astype(np.float32)))
        wv = np.ascontiguousarray(weights[:, vcols]) * WS
        wv1 = _pairize(_q8(wv))
        wv2 = _pairize(_q8(wv - _q8(wv).astype(np.float32)))
        owm = np.ascontiguousarray(
            out_w[np.concatenate([np.arange(64 * h, 64 * h + 64)
                                  for h in heads]), :]).astype(
                                      ml_dtypes.bfloat16)
        m = {
            "x1": x1_by_b[b], "x2": x2_by_b[b],
            "wqk1": wqk1, "wqk2": wqk2, "wv1": wv1, "wv2": wv2,
            "cb8": cb_by_core[core], "id8": idm, "ow": owm,
        }
        if with_qk_bias:
            m["qkb"] = np.ascontiguousarray(
                in_bias[0, 0, np.concatenate([qcols, kcols])].reshape(1, -1)
            ).astype(np.float32) * WS
            m["vb"] = np.ascontiguousarray(
                in_bias[0, 0, vcols].reshape(1, -1)).astype(np.float32) * WS
            m["ones"] = np.ones((1, IC), dtype=np.float32)
        in_maps.append(m)
    return in_maps


def kernel(x, adj, weights, in_bias, out_w, out_bias, gamma):
    x = np.asarray(x, dtype=np.float32)
    adj = np.asarray(adj, dtype=np.float32)
    weights = np.asarray(weights, dtype=np.float32)
    in_bias = np.asarray(in_bias, dtype=np.float32)
    out_w = np.asarray(out_w, dtype=np.float32)
    out_bias = np.asarray(out_bias, dtype=np.float32)
    gamma = np.asarray(gamma, dtype=np.float32)

    with_qk_bias = bool(np.any(in_bias[0, 0, :]))
    key = f"nc_{with_qk_bias}"
    if key not in _cache:
        _cache[key] = _build_program(with_qk_bias)
    nc = _cache[key]

    in_maps = _build_in_maps(x, adj, weights, in_bias, gamma, out_w,
                             with_qk_bias)
    res = bass_utils.run_bass_kernel_spmd(nc, in_maps,
                                          core_ids=list(range(8)),
                                          **RUN_KWARGS)
    _cache["last_result"] = res

    out = np.empty((B, L, H), dtype=np.float32)
    for b in range(B):
        acc = res.results[4 * b]["yT"].astype(np.float32)
        for g in range(1, 4):
            acc += res.results[4 * b + g]["yT"].astype(np.float32)
        out[b] = acc.T + out_bias[0, 0][None, :]
    return out
